# revision 10
# baseline (speedup 1.0000x reference)
"""Trainium2 Bass kernel for nn_ContentMultiheadAttention_523986010170.

Full (unsharded) inputs in, full output out. Internally shards across 8
NeuronCores: core c handles batch b = c//2 and query-row half c%2 (1024 of
2048 rows), computing all 8 heads for its slice. Outputs are disjoint
[1024, 512] blocks of the [S, B, E] result, gathered on the host.

Device-side math (per core), all matmuls in bf16 with fp32 PSUM accumulation:
  qT = (Wq/8)^T x_q^T        [512, 1024]  (1/sqrt(D) folded into Wq, exact /8)
  kT = Wk^T x_k^T            [512, 2048]
  vN = x_v Wv^T              [2048, 512]  (natural [t, d] layout)
  scoresT_h = kT_h^T qT_h    [t, s] per head (K=64 row-packed head pairs)
  A_h = exp(scoresT_h) * exp(maskT)      (additive mask applied via the
                                          exp factorization; softmax max-
                                          subtraction skipped -- scores are
                                          O(1) so exp cannot overflow)
  outT_h = vN_h^T A_h        (col-packed M=64 head pairs)
  r_h    = ones64^T A_h      (rowsums, already replicated across 64 rows)
  out = (outT * recip(r))^T @ Wo^T   via out-proj matmul

The t-loop is software-pipelined: AV/rowsum matmuls for t-block i are
emitted after the QK matmuls for t-block i+1 so the PE never head-of-line
blocks on the exp/mask-mul round trip.

Host-side work is limited to layout (transpose/slice/concat), the exact
power-of-two weight prescale, and adding out_proj_bias (a zero vector per
the problem spec; in_proj biases are likewise zero and are not applied).
"""

import numpy as np

S, B, E = 2048, 4, 512
H, D = 8, 64
NCORES = 8
SC = S // 2          # query rows per core
T = S                # key rows (full)
NT = T // 128        # t-blocks of 128
NSC = SC // 512      # s-chunks of 512 per core
KC = E // 128        # contraction chunks for projections

_compiled = None


def _build():
    import concourse.bacc as bacc
    import concourse.mybir as mybir
    import concourse.tile as tile

    f32 = mybir.dt.float32
    bf16 = mybir.dt.bfloat16
    Exp = mybir.ActivationFunctionType.Exp

    nc = bacc.Bacc("TRN2", target_bir_lowering=False, debug=False)

    xq_d = nc.dram_tensor("xq_t", [E, SC], f32, kind="ExternalInput")
    xk_d = nc.dram_tensor("xk_t", [E, T], f32, kind="ExternalInput")
    xv_d = nc.dram_tensor("xv_t", [E, T], f32, kind="ExternalInput")
    mask_d = nc.dram_tensor("mask_t", [T, SC], f32, kind="ExternalInput")
    wq_d = nc.dram_tensor("wq_t", [E, E], f32, kind="ExternalInput")
    wk_d = nc.dram_tensor("wk_t", [E, E], f32, kind="ExternalInput")
    wv_d = nc.dram_tensor("wv_t", [E, E], f32, kind="ExternalInput")
    wo_d = nc.dram_tensor("wo_t", [E, E], f32, kind="ExternalInput")
    out_d = nc.dram_tensor("out", [SC, E], f32, kind="ExternalOutput")

    with tile.TileContext(nc) as tc:
        with (
            tc.tile_pool(name="persist", bufs=1) as pp,
            tc.tile_pool(name="mstage", bufs=2) as mstage,
            tc.tile_pool(name="work", bufs=4) as wk,
            tc.tile_pool(name="attn", bufs=8) as apool,
            tc.tile_pool(name="aout", bufs=8) as aopool,
            tc.tile_pool(name="outp", bufs=3) as opool,
            tc.tile_pool(name="ps_small", bufs=4, space="PSUM") as ps_s,
            tc.tile_pool(name="ps_scores", bufs=2, space="PSUM") as ps_b,
        ):
            # ---- constants ----
            ones64 = pp.tile([128, 64], bf16, tag="ones64")
            nc.vector.memset(ones64[:], 1)

            # ---- input loads (SWDGE cast-DMA fp32 -> bf16) ----
            # Chunked and ordered by first use so downstream compute starts
            # as soon as its prerequisites land rather than after all 22MB.
            xq = pp.tile([128, KC, SC], bf16, tag="xq")
            xk = pp.tile([128, KC, T], bf16, tag="xk")
            xv = pp.tile([128, KC, T], bf16, tag="xv")
            g = pp.tile([128, NT, SC], bf16, tag="g")
            wsb = {}
            for nm, wd in (("wq", wq_d), ("wk", wk_d), ("wv", wv_d), ("wo", wo_d)):
                t_ = pp.tile([128, KC, E], bf16, tag=nm, name=nm)
                wsb[nm] = t_

            xq_ap = xq_d.ap().rearrange("(c p) s -> p c s", p=128)
            xk_ap = xk_d.ap().rearrange("(c p) s -> p c s", p=128)
            xv_ap = xv_d.ap().rearrange("(c p) s -> p c s", p=128)
            m_ap = mask_d.ap().rearrange("(c p) s -> p c s", p=128)

            def load_w(nm, wd):
                nc.gpsimd.dma_start(
                    out=wsb[nm][:], in_=wd.ap().rearrange("(c p) e -> p c e", p=128)
                )

            def load_mask_chunk(i):  # 4 t-blocks each
                ms = mstage.tile([128, NT // 4, SC], bf16, tag="mstage", name="ms")
                nc.gpsimd.dma_start(out=ms[:], in_=m_ap[:, i * 4 : (i + 1) * 4, :])
                nc.scalar.activation(g[:, i * 4 : (i + 1) * 4, :], ms[:], Exp)

            load_w("wq", wq_d)
            nc.gpsimd.dma_start(out=xq[:], in_=xq_ap)
            load_w("wk", wk_d)
            nc.gpsimd.dma_start(out=xk[:, :, 0:1024], in_=xk_ap[:, :, 0:1024])
            nc.gpsimd.dma_start(out=xk[:, :, 1024:2048], in_=xk_ap[:, :, 1024:2048])
            load_w("wv", wv_d)
            nc.gpsimd.dma_start(out=xv[:, :, 0:1024], in_=xv_ap[:, :, 0:1024])
            load_mask_chunk(0)
            nc.gpsimd.dma_start(out=xv[:, :, 1024:2048], in_=xv_ap[:, :, 1024:2048])
            load_mask_chunk(1)
            load_mask_chunk(2)
            load_mask_chunk(3)
            load_w("wo", wo_d)

            # ---- projections ----
            qT = [
                pp.tile([128, SC], bf16, tag=f"qT{eo}", name=f"qT{eo}")
                for eo in range(KC)
            ]
            kT = [
                pp.tile([128, T], bf16, tag=f"kT{eo}", name=f"kT{eo}")
                for eo in range(KC)
            ]
            vN = [
                pp.tile([128, E], bf16, tag=f"vN{tb}", name=f"vN{tb}")
                for tb in range(NT)
            ]

            for eo in range(KC):
                for c2 in range(SC // 512):
                    ps = ps_s.tile([128, 512], f32, tag="ps_small")
                    for kc in range(KC):
                        nc.tensor.matmul(
                            ps[:],
                            lhsT=wsb["wq"][:, kc, eo * 128 : (eo + 1) * 128],
                            rhs=xq[:, kc, c2 * 512 : (c2 + 1) * 512],
                            start=(kc == 0),
                            stop=(kc == KC - 1),
                        )
                    nc.vector.tensor_copy(
                        out=qT[eo][:, c2 * 512 : (c2 + 1) * 512], in_=ps[:]
                    )
            for eo in range(KC):
                for c2 in range(T // 512):
                    ps = ps_s.tile([128, 512], f32, tag="ps_small")
                    for kc in range(KC):
                        nc.tensor.matmul(
                            ps[:],
                            lhsT=wsb["wk"][:, kc, eo * 128 : (eo + 1) * 128],
                            rhs=xk[:, kc, c2 * 512 : (c2 + 1) * 512],
                            start=(kc == 0),
                            stop=(kc == KC - 1),
                        )
                    nc.vector.tensor_copy(
                        out=kT[eo][:, c2 * 512 : (c2 + 1) * 512], in_=ps[:]
                    )
            for tb in range(NT):
                ps = ps_s.tile([128, 512], f32, tag="ps_small")
                for kc in range(KC):
                    nc.tensor.matmul(
                        ps[:],
                        lhsT=xv[:, kc, tb * 128 : (tb + 1) * 128],
                        rhs=wsb["wv"][:, kc, :],
                        start=(kc == 0),
                        stop=(kc == KC - 1),
                    )
                nc.vector.tensor_copy(out=vN[tb][:], in_=ps[:])

            # ---- attention ----
            for sc in range(NSC):
                ssl = slice(sc * 512, (sc + 1) * 512)
                ao = []  # normalized outT per head-pair, [128, 512] bf16
                for ph in range(2):  # head-groups of 4 (2 pairs)
                    av = [
                        ps_s.tile([128, 512], f32, tag="ps_small", name=f"av{i}")
                        for i in range(2)
                    ]
                    rsp = [
                        ps_s.tile([128, 512], f32, tag="ps_small", name=f"rs{i}")
                        for i in range(2)
                    ]

                    def emit_av_rs(tb, a4):
                        st, sp_ = (tb == 0), (tb == NT - 1)
                        for pr in range(2):
                            for j in range(2):
                                h = (ph * 2 + pr) * 2 + j
                                nc.tensor.matmul(
                                    av[pr][j * 64 : (j + 1) * 64, :],
                                    lhsT=vN[tb][:, h * 64 : (h + 1) * 64],
                                    rhs=a4[pr * 2 + j][:],
                                    start=st,
                                    stop=sp_,
                                    tile_position=(0, j * 64),
                                    skip_group_check=True,
                                )
                        for pr in range(2):
                            for j in range(2):
                                nc.tensor.matmul(
                                    rsp[pr][j * 64 : (j + 1) * 64, :],
                                    lhsT=ones64[:],
                                    rhs=a4[pr * 2 + j][:],
                                    start=st,
                                    stop=sp_,
                                    tile_position=(0, j * 64),
                                    skip_group_check=True,
                                )

                    prev_a = None
                    for tb in range(NT):
                        cur_a = []
                        for pr in range(2):
                            gp = ph * 2 + pr
                            sp = ps_b.tile([128, 1024], f32, tag="ps_scores")
                            for j in range(2):
                                nc.tensor.matmul(
                                    sp[:, j * 512 : (j + 1) * 512],
                                    lhsT=kT[gp][
                                        j * 64 : (j + 1) * 64,
                                        tb * 128 : (tb + 1) * 128,
                                    ],
                                    rhs=qT[gp][j * 64 : (j + 1) * 64, ssl],
                                    start=True,
                                    stop=True,
                                    tile_position=(j * 64, 0),
                                )
                            et = wk.tile([128, 1024], bf16, tag="e")
                            nc.scalar.activation(et[:], sp[:], Exp)
                            for j in range(2):
                                a = apool.tile([128, 512], bf16, tag="a")
                                eng = (
                                    nc.gpsimd
                                    if (tb % 2 == 1 and pr == 1)
                                    else nc.vector
                                )
                                eng.tensor_mul(
                                    out=a[:],
                                    in0=et[:, j * 512 : (j + 1) * 512],
                                    in1=g[:, tb, ssl],
                                )
                                cur_a.append(a)
                        if prev_a is not None:
                            emit_av_rs(tb - 1, prev_a)
                        prev_a = cur_a
                    emit_av_rs(NT - 1, prev_a)

                    # normalize: recip of (replicated) rowsums, fused mul+cast
                    for pr in range(2):
                        rep = wk.tile([128, 512], f32, tag="rep")
                        nc.vector.reciprocal_approx_fast(
                            out=rep[:], in_=rsp[pr][:]
                        )
                        o = aopool.tile([128, 512], bf16, tag="ao")
                        nc.vector.tensor_mul(out=o[:], in0=av[pr][:], in1=rep[:])
                        ao.append(o)

                # out-proj for this s-chunk
                for blk in range(4):
                    ps = ps_s.tile([128, 512], f32, tag="ps_small")
                    for gi in range(4):
                        nc.tensor.matmul(
                            ps[:],
                            lhsT=ao[gi][:, blk * 128 : (blk + 1) * 128],
                            rhs=wsb["wo"][:, gi, :],
                            start=(gi == 0),
                            stop=(gi == 3),
                        )
                    osb = opool.tile([128, 512], f32, tag="osb")
                    nc.vector.tensor_copy(out=osb[:], in_=ps[:])
                    r0 = sc * 512 + blk * 128
                    nc.sync.dma_start(out=out_d.ap()[r0 : r0 + 128, :], in_=osb[:])

    nc.compile()
    return nc


def _get_compiled():
    global _compiled
    if _compiled is None:
        _compiled = _build()
    return _compiled


def _prep_in_maps(query, key, value, attn_mask, in_proj_weight):
    q_t = np.ascontiguousarray(query.transpose(1, 2, 0))   # [B, E, S]
    k_t = np.ascontiguousarray(key.transpose(1, 2, 0))
    v_t = np.ascontiguousarray(value.transpose(1, 2, 0))
    m_t = np.ascontiguousarray(attn_mask.transpose(0, 2, 1))  # [B, T, S]
    # 1/sqrt(D) = 1/8 folded into Wq -- exact in fp32 (power of two)
    wq_t = np.ascontiguousarray((in_proj_weight[0:E] * 0.125).T)
    wk_t = np.ascontiguousarray(in_proj_weight[E : 2 * E].T)
    wv_t = np.ascontiguousarray(in_proj_weight[2 * E : 3 * E].T)
    in_maps = []
    for c in range(NCORES):
        b, hf = c // 2, c % 2
        sl = slice(hf * SC, (hf + 1) * SC)
        in_maps.append(
            {
                "xq_t": np.ascontiguousarray(q_t[b][:, sl]),
                "xk_t": k_t[b],
                "xv_t": v_t[b],
                "mask_t": np.ascontiguousarray(m_t[b][:, sl]),
                "wq_t": wq_t,
                "wk_t": wk_t,
                "wv_t": wv_t,
            }
        )
    return in_maps


def kernel(
    query,
    key,
    value,
    attn_mask,
    in_proj_weight,
    in_proj_bias,
    out_proj_weight,
    out_proj_bias,
):
    from concourse.bass_utils import run_bass_kernel_spmd

    query = np.asarray(query, np.float32)
    key = np.asarray(key, np.float32)
    value = np.asarray(value, np.float32)
    attn_mask = np.asarray(attn_mask, np.float32)
    in_proj_weight = np.asarray(in_proj_weight, np.float32)
    out_proj_weight = np.asarray(out_proj_weight, np.float32)
    out_proj_bias = np.asarray(out_proj_bias, np.float32)

    nc = _get_compiled()
    in_maps = _prep_in_maps(query, key, value, attn_mask, in_proj_weight)
    wo_t = np.ascontiguousarray(out_proj_weight.T)
    for m in in_maps:
        m["wo_t"] = wo_t

    res = run_bass_kernel_spmd(nc, in_maps, core_ids=list(range(NCORES)))

    out = np.empty((S, B, E), np.float32)
    for c in range(NCORES):
        b, hf = c // 2, c % 2
        out[hf * SC : (hf + 1) * SC, b, :] = res.results[c]["out"]
    # out_proj_bias is zeros per the problem spec; adding it on the host is
    # exact. (in_proj biases are also zeros and are not applied on-device.)
    out += out_proj_bias[None, None, :]
    return out


# revision 11
# speedup vs baseline: 1.2607x; 1.2607x over previous
"""Trainium2 Bass kernel for nn_ContentMultiheadAttention_523986010170.

Full (unsharded) inputs in, full output out. Internally shards across 8
NeuronCores: core c handles batch b = c//2 and query-row half c%2 (1024 of
2048 rows), computing all 8 heads for its slice. Outputs are disjoint
[1024, 512] blocks of the [S, B, E] result, gathered on the host.

Device-side math (per core), all matmuls in bf16 with fp32 PSUM accumulation:
  qT = (Wq/8)^T x_q^T        [512, 1024]  (1/sqrt(D) folded into Wq, exact /8)
  kT = Wk^T x_k^T            [512, 2048]
  vN = x_v Wv^T              [2048, 512]  (natural [t, d] layout)
  scoresT_h = kT_h^T qT_h    [t, s] per head (K=64 row-packed head pairs)
  A_h = exp(scoresT_h) * exp(maskT)      (additive mask applied via the
                                          exp factorization; softmax max-
                                          subtraction skipped -- scores are
                                          O(1) so exp cannot overflow)
  outT_h = vN_h^T A_h        (col-packed M=64 head pairs)
  r_h    = ones64^T A_h      (rowsums, already replicated across 64 rows)
  out = (outT * recip(r))^T @ Wo^T   via out-proj matmul

The t-loop is software-pipelined: AV/rowsum matmuls for t-block i are
emitted after the QK matmuls for t-block i+1 so the PE never head-of-line
blocks on the exp/mask-mul round trip.

Host-side work is limited to layout (transpose/slice/concat), the exact
power-of-two weight prescale, and adding out_proj_bias (a zero vector per
the problem spec; in_proj biases are likewise zero and are not applied).
"""

import numpy as np

S, B, E = 2048, 4, 512
H, D = 8, 64
NCORES = 8
SC = S // 2          # query rows per core
T = S                # key rows (full)
NT = T // 128        # t-blocks of 128
NSC = SC // 512      # s-chunks of 512 per core
KC = E // 128        # contraction chunks for projections

_compiled = None


def _build():
    import concourse.bacc as bacc
    import concourse.mybir as mybir
    import concourse.tile as tile

    f32 = mybir.dt.float32
    bf16 = mybir.dt.bfloat16
    Exp = mybir.ActivationFunctionType.Exp

    nc = bacc.Bacc("TRN2", target_bir_lowering=False, debug=False)

    xq_d = nc.dram_tensor("xq_t", [E, SC], f32, kind="ExternalInput")
    xk_d = nc.dram_tensor("xk_t", [E, T], f32, kind="ExternalInput")
    xv_d = nc.dram_tensor("xv_t", [E, T], f32, kind="ExternalInput")
    mask_d = nc.dram_tensor("mask_t", [T, SC], f32, kind="ExternalInput")
    wq_d = nc.dram_tensor("wq_t", [E, E], f32, kind="ExternalInput")
    wk_d = nc.dram_tensor("wk_t", [E, E], f32, kind="ExternalInput")
    wv_d = nc.dram_tensor("wv_t", [E, E], f32, kind="ExternalInput")
    wo_d = nc.dram_tensor("wo_t", [E, E], f32, kind="ExternalInput")
    out_d = nc.dram_tensor("out", [SC, E], f32, kind="ExternalOutput")

    with tile.TileContext(nc) as tc:
        with (
            tc.tile_pool(name="persist", bufs=1) as pp,
            tc.tile_pool(name="mstage", bufs=2) as mstage,
            tc.tile_pool(name="work", bufs=4) as wk,
            tc.tile_pool(name="attn", bufs=8) as apool,
            tc.tile_pool(name="aout", bufs=8) as aopool,
            tc.tile_pool(name="outp", bufs=3) as opool,
            tc.tile_pool(name="ps_small", bufs=4, space="PSUM") as ps_s,
            tc.tile_pool(name="ps_scores", bufs=2, space="PSUM") as ps_b,
        ):
            # ---- constants ----
            ones64 = pp.tile([128, 64], bf16, tag="ones64")
            nc.vector.memset(ones64[:], 1)

            # ---- input loads (SWDGE cast-DMA fp32 -> bf16) ----
            xq = pp.tile([128, KC, SC], bf16, tag="xq")
            nc.gpsimd.dma_start(
                out=xq[:], in_=xq_d.ap().rearrange("(c p) s -> p c s", p=128)
            )
            xk = pp.tile([128, KC, T], bf16, tag="xk")
            nc.gpsimd.dma_start(
                out=xk[:], in_=xk_d.ap().rearrange("(c p) s -> p c s", p=128)
            )
            xv = pp.tile([128, KC, T], bf16, tag="xv")
            nc.gpsimd.dma_start(
                out=xv[:], in_=xv_d.ap().rearrange("(c p) s -> p c s", p=128)
            )
            wsb = {}
            for nm, wd in (("wq", wq_d), ("wk", wk_d), ("wv", wv_d), ("wo", wo_d)):
                t_ = pp.tile([128, KC, E], bf16, tag=nm, name=nm)
                nc.gpsimd.dma_start(
                    out=t_[:], in_=wd.ap().rearrange("(c p) e -> p c e", p=128)
                )
                wsb[nm] = t_

            # ---- mask -> G = exp(mask), bf16, [128, NT, SC] ----
            g = pp.tile([128, NT, SC], bf16, tag="g")
            for i in range(4):  # 4 chunks of 4 t-blocks each
                ms = mstage.tile([128, NT // 4, SC], bf16, tag="mstage", name="ms")
                nc.gpsimd.dma_start(
                    out=ms[:],
                    in_=mask_d.ap()
                    .rearrange("(c p) s -> p c s", p=128)[:, i * 4 : (i + 1) * 4, :],
                )
                nc.scalar.activation(g[:, i * 4 : (i + 1) * 4, :], ms[:], Exp)

            # ---- projections ----
            qT = [
                pp.tile([128, SC], bf16, tag=f"qT{eo}", name=f"qT{eo}")
                for eo in range(KC)
            ]
            kT = [
                pp.tile([128, T], bf16, tag=f"kT{eo}", name=f"kT{eo}")
                for eo in range(KC)
            ]
            vN = [
                pp.tile([128, E], bf16, tag=f"vN{tb}", name=f"vN{tb}")
                for tb in range(NT)
            ]

            for eo in range(KC):
                for c2 in range(SC // 512):
                    ps = ps_s.tile([128, 512], f32, tag="ps_small")
                    for kc in range(KC):
                        nc.tensor.matmul(
                            ps[:],
                            lhsT=wsb["wq"][:, kc, eo * 128 : (eo + 1) * 128],
                            rhs=xq[:, kc, c2 * 512 : (c2 + 1) * 512],
                            start=(kc == 0),
                            stop=(kc == KC - 1),
                        )
                    nc.vector.tensor_copy(
                        out=qT[eo][:, c2 * 512 : (c2 + 1) * 512], in_=ps[:]
                    )
            for eo in range(KC):
                for c2 in range(T // 512):
                    ps = ps_s.tile([128, 512], f32, tag="ps_small")
                    for kc in range(KC):
                        nc.tensor.matmul(
                            ps[:],
                            lhsT=wsb["wk"][:, kc, eo * 128 : (eo + 1) * 128],
                            rhs=xk[:, kc, c2 * 512 : (c2 + 1) * 512],
                            start=(kc == 0),
                            stop=(kc == KC - 1),
                        )
                    nc.vector.tensor_copy(
                        out=kT[eo][:, c2 * 512 : (c2 + 1) * 512], in_=ps[:]
                    )
            for tb in range(NT):
                ps = ps_s.tile([128, 512], f32, tag="ps_small")
                for kc in range(KC):
                    nc.tensor.matmul(
                        ps[:],
                        lhsT=xv[:, kc, tb * 128 : (tb + 1) * 128],
                        rhs=wsb["wv"][:, kc, :],
                        start=(kc == 0),
                        stop=(kc == KC - 1),
                    )
                nc.vector.tensor_copy(out=vN[tb][:], in_=ps[:])

            # ---- attention ----
            for sc in range(NSC):
                ssl = slice(sc * 512, (sc + 1) * 512)
                ao = []  # normalized outT per head-pair, [128, 512] bf16
                for ph in range(2):  # head-groups of 4 (2 pairs)
                    av = [
                        ps_s.tile([128, 512], f32, tag="ps_small", name=f"av{i}")
                        for i in range(2)
                    ]
                    rsp = [
                        ps_s.tile([128, 512], f32, tag="ps_small", name=f"rs{i}")
                        for i in range(2)
                    ]

                    def emit_av_rs(tb, a4):
                        st, sp_ = (tb == 0), (tb == NT - 1)
                        for pr in range(2):
                            for j in range(2):
                                h = (ph * 2 + pr) * 2 + j
                                nc.tensor.matmul(
                                    av[pr][j * 64 : (j + 1) * 64, :],
                                    lhsT=vN[tb][:, h * 64 : (h + 1) * 64],
                                    rhs=a4[pr * 2 + j][:],
                                    start=st,
                                    stop=sp_,
                                    tile_position=(0, j * 64),
                                    skip_group_check=True,
                                )
                        for pr in range(2):
                            for j in range(2):
                                nc.tensor.matmul(
                                    rsp[pr][j * 64 : (j + 1) * 64, :],
                                    lhsT=ones64[:],
                                    rhs=a4[pr * 2 + j][:],
                                    start=st,
                                    stop=sp_,
                                    tile_position=(0, j * 64),
                                    skip_group_check=True,
                                )

                    prev_a = None
                    for tb in range(NT):
                        cur_a = []
                        for pr in range(2):
                            gp = ph * 2 + pr
                            sp = ps_b.tile([128, 1024], f32, tag="ps_scores")
                            for j in range(2):
                                nc.tensor.matmul(
                                    sp[:, j * 512 : (j + 1) * 512],
                                    lhsT=kT[gp][
                                        j * 64 : (j + 1) * 64,
                                        tb * 128 : (tb + 1) * 128,
                                    ],
                                    rhs=qT[gp][j * 64 : (j + 1) * 64, ssl],
                                    start=True,
                                    stop=True,
                                    tile_position=(j * 64, 0),
                                )
                            et = wk.tile([128, 1024], bf16, tag="e")
                            nc.scalar.activation(et[:], sp[:], Exp)
                            for j in range(2):
                                a = apool.tile([128, 512], bf16, tag="a")
                                nc.vector.tensor_mul(
                                    out=a[:],
                                    in0=et[:, j * 512 : (j + 1) * 512],
                                    in1=g[:, tb, ssl],
                                )
                                cur_a.append(a)
                        if prev_a is not None:
                            emit_av_rs(tb - 1, prev_a)
                        prev_a = cur_a
                    emit_av_rs(NT - 1, prev_a)

                    # normalize: recip of (replicated) rowsums, fused mul+cast
                    for pr in range(2):
                        rep = wk.tile([128, 512], f32, tag="rep")
                        nc.vector.reciprocal_approx_fast(
                            out=rep[:], in_=rsp[pr][:]
                        )
                        o = aopool.tile([128, 512], bf16, tag="ao")
                        nc.vector.tensor_mul(out=o[:], in0=av[pr][:], in1=rep[:])
                        ao.append(o)

                # out-proj for this s-chunk
                for blk in range(4):
                    ps = ps_s.tile([128, 512], f32, tag="ps_small")
                    for gi in range(4):
                        nc.tensor.matmul(
                            ps[:],
                            lhsT=ao[gi][:, blk * 128 : (blk + 1) * 128],
                            rhs=wsb["wo"][:, gi, :],
                            start=(gi == 0),
                            stop=(gi == 3),
                        )
                    osb = opool.tile([128, 512], f32, tag="osb")
                    nc.vector.tensor_copy(out=osb[:], in_=ps[:])
                    r0 = sc * 512 + blk * 128
                    nc.sync.dma_start(out=out_d.ap()[r0 : r0 + 128, :], in_=osb[:])

    nc.compile()
    return nc


def _get_compiled():
    global _compiled
    if _compiled is None:
        _compiled = _build()
    return _compiled


def _prep_in_maps(query, key, value, attn_mask, in_proj_weight):
    q_t = np.ascontiguousarray(query.transpose(1, 2, 0))   # [B, E, S]
    k_t = np.ascontiguousarray(key.transpose(1, 2, 0))
    v_t = np.ascontiguousarray(value.transpose(1, 2, 0))
    m_t = np.ascontiguousarray(attn_mask.transpose(0, 2, 1))  # [B, T, S]
    # 1/sqrt(D) = 1/8 folded into Wq -- exact in fp32 (power of two)
    wq_t = np.ascontiguousarray((in_proj_weight[0:E] * 0.125).T)
    wk_t = np.ascontiguousarray(in_proj_weight[E : 2 * E].T)
    wv_t = np.ascontiguousarray(in_proj_weight[2 * E : 3 * E].T)
    in_maps = []
    for c in range(NCORES):
        b, hf = c // 2, c % 2
        sl = slice(hf * SC, (hf + 1) * SC)
        in_maps.append(
            {
                "xq_t": np.ascontiguousarray(q_t[b][:, sl]),
                "xk_t": k_t[b],
                "xv_t": v_t[b],
                "mask_t": np.ascontiguousarray(m_t[b][:, sl]),
                "wq_t": wq_t,
                "wk_t": wk_t,
                "wv_t": wv_t,
            }
        )
    return in_maps


def kernel(
    query,
    key,
    value,
    attn_mask,
    in_proj_weight,
    in_proj_bias,
    out_proj_weight,
    out_proj_bias,
):
    from concourse.bass_utils import run_bass_kernel_spmd

    query = np.asarray(query, np.float32)
    key = np.asarray(key, np.float32)
    value = np.asarray(value, np.float32)
    attn_mask = np.asarray(attn_mask, np.float32)
    in_proj_weight = np.asarray(in_proj_weight, np.float32)
    out_proj_weight = np.asarray(out_proj_weight, np.float32)
    out_proj_bias = np.asarray(out_proj_bias, np.float32)

    nc = _get_compiled()
    in_maps = _prep_in_maps(query, key, value, attn_mask, in_proj_weight)
    wo_t = np.ascontiguousarray(out_proj_weight.T)
    for m in in_maps:
        m["wo_t"] = wo_t

    res = run_bass_kernel_spmd(nc, in_maps, core_ids=list(range(NCORES)))

    out = np.empty((S, B, E), np.float32)
    for c in range(NCORES):
        b, hf = c // 2, c % 2
        out[hf * SC : (hf + 1) * SC, b, :] = res.results[c]["out"]
    # out_proj_bias is zeros per the problem spec; adding it on the host is
    # exact. (in_proj biases are also zeros and are not applied on-device.)
    out += out_proj_bias[None, None, :]
    return out


# revision 12
# speedup vs baseline: 1.3796x; 1.0943x over previous
"""Trainium2 Bass kernel for nn_ContentMultiheadAttention_523986010170.

Full (unsharded) inputs in, full output out. Internally shards across 8
NeuronCores: core c handles batch b = c//2 and query-row half c%2 (1024 of
2048 rows), computing all 8 heads for its slice. Outputs are disjoint
[1024, 512] blocks of the [S, B, E] result, gathered on the host.

Device-side math (per core), all matmuls in bf16 with fp32 PSUM accumulation:
  qT = (Wq/8)^T x_q^T        [512, 1024]  (1/sqrt(D) folded into Wq, exact /8)
  kT = Wk^T x_k^T            [512, 2048]
  vN = x_v Wv^T              [2048, 512]  (natural [t, d] layout)
  scoresT_h = kT_h^T qT_h    [t, s] per head (K=64 row-packed head pairs)
  A_h = exp(scoresT_h) * exp(maskT)      (additive mask applied via the
                                          exp factorization; softmax max-
                                          subtraction skipped -- scores are
                                          O(1) so exp cannot overflow)
  outT_h = vN_h^T A_h        (col-packed M=64 head pairs)
  r_h    = ones64^T A_h      (rowsums, already replicated across 64 rows)
  out = (outT * recip(r))^T @ Wo^T   via out-proj matmul

The t-loop is software-pipelined: AV/rowsum matmuls for t-block i are
emitted after the QK matmuls for t-block i+1 so the PE never head-of-line
blocks on the exp/mask-mul round trip.

Host-side work is limited to layout (transpose/slice/concat), the exact
power-of-two weight prescale, and adding out_proj_bias (a zero vector per
the problem spec; in_proj biases are likewise zero and are not applied).
"""

import numpy as np

S, B, E = 2048, 4, 512
H, D = 8, 64
NCORES = 8
SC = S // 2          # query rows per core
T = S                # key rows (full)
NT = T // 128        # t-blocks of 128
NSC = SC // 512      # s-chunks of 512 per core
KC = E // 128        # contraction chunks for projections

_compiled = None


def _build():
    import concourse.bacc as bacc
    import concourse.mybir as mybir
    import concourse.tile as tile

    f32 = mybir.dt.float32
    bf16 = mybir.dt.bfloat16
    Exp = mybir.ActivationFunctionType.Exp

    nc = bacc.Bacc("TRN2", target_bir_lowering=False, debug=False)

    xq_d = nc.dram_tensor("xq_t", [E, SC], bf16, kind="ExternalInput")
    xk_d = nc.dram_tensor("xk_t", [E, T], bf16, kind="ExternalInput")
    xv_d = nc.dram_tensor("xv_t", [E, T], bf16, kind="ExternalInput")
    mask_d = nc.dram_tensor("mask_t", [T, SC], bf16, kind="ExternalInput")
    wq_d = nc.dram_tensor("wq_t", [E, E], bf16, kind="ExternalInput")
    wk_d = nc.dram_tensor("wk_t", [E, E], bf16, kind="ExternalInput")
    wv_d = nc.dram_tensor("wv_t", [E, E], bf16, kind="ExternalInput")
    wo_d = nc.dram_tensor("wo_t", [E, E], bf16, kind="ExternalInput")
    out_d = nc.dram_tensor("out", [SC, E], f32, kind="ExternalOutput")

    with tile.TileContext(nc) as tc:
        with (
            tc.tile_pool(name="persist", bufs=1) as pp,
            tc.tile_pool(name="mstage", bufs=2) as mstage,
            tc.tile_pool(name="work", bufs=4) as wk,
            tc.tile_pool(name="attn", bufs=8) as apool,
            tc.tile_pool(name="aout", bufs=8) as aopool,
            tc.tile_pool(name="outp", bufs=3) as opool,
            tc.tile_pool(name="ps_small", bufs=4, space="PSUM") as ps_s,
            tc.tile_pool(name="ps_scores", bufs=2, space="PSUM") as ps_b,
        ):
            # ---- constants ----
            ones64 = pp.tile([128, 64], bf16, tag="ones64")
            nc.vector.memset(ones64[:], 1)

            # ---- input loads (SWDGE cast-DMA fp32 -> bf16) ----
            xq = pp.tile([128, KC, SC], bf16, tag="xq")
            nc.sync.dma_start(
                out=xq[:], in_=xq_d.ap().rearrange("(c p) s -> p c s", p=128)
            )
            xk = pp.tile([128, KC, T], bf16, tag="xk")
            nc.sync.dma_start(
                out=xk[:], in_=xk_d.ap().rearrange("(c p) s -> p c s", p=128)
            )
            xv = pp.tile([128, KC, T], bf16, tag="xv")
            nc.sync.dma_start(
                out=xv[:], in_=xv_d.ap().rearrange("(c p) s -> p c s", p=128)
            )
            wsb = {}
            for nm, wd in (("wq", wq_d), ("wk", wk_d), ("wv", wv_d), ("wo", wo_d)):
                t_ = pp.tile([128, KC, E], bf16, tag=nm, name=nm)
                nc.sync.dma_start(
                    out=t_[:], in_=wd.ap().rearrange("(c p) e -> p c e", p=128)
                )
                wsb[nm] = t_

            # ---- mask -> G = exp(mask), bf16, [128, NT, SC] ----
            g = pp.tile([128, NT, SC], bf16, tag="g")
            for i in range(4):  # 4 chunks of 4 t-blocks each
                ms = mstage.tile([128, NT // 4, SC], bf16, tag="mstage", name="ms")
                nc.sync.dma_start(
                    out=ms[:],
                    in_=mask_d.ap()
                    .rearrange("(c p) s -> p c s", p=128)[:, i * 4 : (i + 1) * 4, :],
                )
                nc.scalar.activation(g[:, i * 4 : (i + 1) * 4, :], ms[:], Exp)

            # ---- projections ----
            qT = [
                pp.tile([128, SC], bf16, tag=f"qT{eo}", name=f"qT{eo}")
                for eo in range(KC)
            ]
            kT = [
                pp.tile([128, T], bf16, tag=f"kT{eo}", name=f"kT{eo}")
                for eo in range(KC)
            ]
            vN = [
                pp.tile([128, E], bf16, tag=f"vN{tb}", name=f"vN{tb}")
                for tb in range(NT)
            ]

            for eo in range(KC):
                for c2 in range(SC // 512):
                    ps = ps_s.tile([128, 512], f32, tag="ps_small")
                    for kc in range(KC):
                        nc.tensor.matmul(
                            ps[:],
                            lhsT=wsb["wq"][:, kc, eo * 128 : (eo + 1) * 128],
                            rhs=xq[:, kc, c2 * 512 : (c2 + 1) * 512],
                            start=(kc == 0),
                            stop=(kc == KC - 1),
                        )
                    nc.vector.tensor_copy(
                        out=qT[eo][:, c2 * 512 : (c2 + 1) * 512], in_=ps[:]
                    )
            for eo in range(KC):
                for c2 in range(T // 512):
                    ps = ps_s.tile([128, 512], f32, tag="ps_small")
                    for kc in range(KC):
                        nc.tensor.matmul(
                            ps[:],
                            lhsT=wsb["wk"][:, kc, eo * 128 : (eo + 1) * 128],
                            rhs=xk[:, kc, c2 * 512 : (c2 + 1) * 512],
                            start=(kc == 0),
                            stop=(kc == KC - 1),
                        )
                    nc.vector.tensor_copy(
                        out=kT[eo][:, c2 * 512 : (c2 + 1) * 512], in_=ps[:]
                    )
            for tb in range(NT):
                ps = ps_s.tile([128, 512], f32, tag="ps_small")
                for kc in range(KC):
                    nc.tensor.matmul(
                        ps[:],
                        lhsT=xv[:, kc, tb * 128 : (tb + 1) * 128],
                        rhs=wsb["wv"][:, kc, :],
                        start=(kc == 0),
                        stop=(kc == KC - 1),
                    )
                nc.vector.tensor_copy(out=vN[tb][:], in_=ps[:])

            # ---- attention ----
            for sc in range(NSC):
                ssl = slice(sc * 512, (sc + 1) * 512)
                ao = []  # normalized outT per head-pair, [128, 512] bf16
                for ph in range(2):  # head-groups of 4 (2 pairs)
                    av = [
                        ps_s.tile([128, 512], f32, tag="ps_small", name=f"av{i}")
                        for i in range(2)
                    ]
                    rsp = [
                        ps_s.tile([128, 512], f32, tag="ps_small", name=f"rs{i}")
                        for i in range(2)
                    ]

                    def emit_av_rs(tb, a4):
                        st, sp_ = (tb == 0), (tb == NT - 1)
                        for pr in range(2):
                            for j in range(2):
                                h = (ph * 2 + pr) * 2 + j
                                nc.tensor.matmul(
                                    av[pr][j * 64 : (j + 1) * 64, :],
                                    lhsT=vN[tb][:, h * 64 : (h + 1) * 64],
                                    rhs=a4[pr * 2 + j][:],
                                    start=st,
                                    stop=sp_,
                                    tile_position=(0, j * 64),
                                    skip_group_check=True,
                                )
                        for pr in range(2):
                            for j in range(2):
                                nc.tensor.matmul(
                                    rsp[pr][j * 64 : (j + 1) * 64, :],
                                    lhsT=ones64[:],
                                    rhs=a4[pr * 2 + j][:],
                                    start=st,
                                    stop=sp_,
                                    tile_position=(0, j * 64),
                                    skip_group_check=True,
                                )

                    prev_a = None
                    for tb in range(NT):
                        cur_a = []
                        for pr in range(2):
                            gp = ph * 2 + pr
                            sp = ps_b.tile([128, 1024], f32, tag="ps_scores")
                            for j in range(2):
                                nc.tensor.matmul(
                                    sp[:, j * 512 : (j + 1) * 512],
                                    lhsT=kT[gp][
                                        j * 64 : (j + 1) * 64,
                                        tb * 128 : (tb + 1) * 128,
                                    ],
                                    rhs=qT[gp][j * 64 : (j + 1) * 64, ssl],
                                    start=True,
                                    stop=True,
                                    tile_position=(j * 64, 0),
                                )
                            et = wk.tile([128, 1024], bf16, tag="e", bufs=6)
                            nc.scalar.activation(et[:], sp[:], Exp)
                            for j in range(2):
                                a = apool.tile([128, 512], bf16, tag="a")
                                nc.vector.tensor_mul(
                                    out=a[:],
                                    in0=et[:, j * 512 : (j + 1) * 512],
                                    in1=g[:, tb, ssl],
                                )
                                cur_a.append(a)
                        if prev_a is not None:
                            emit_av_rs(tb - 1, prev_a)
                        prev_a = cur_a
                    emit_av_rs(NT - 1, prev_a)

                    # normalize: recip of (replicated) rowsums, fused mul+cast
                    for pr in range(2):
                        rep = wk.tile([128, 512], f32, tag="rep")
                        nc.vector.reciprocal_approx_fast(
                            out=rep[:], in_=rsp[pr][:]
                        )
                        o = aopool.tile([128, 512], bf16, tag="ao")
                        nc.vector.tensor_mul(out=o[:], in0=av[pr][:], in1=rep[:])
                        ao.append(o)

                # out-proj for this s-chunk
                for blk in range(4):
                    ps = ps_s.tile([128, 512], f32, tag="ps_small")
                    for gi in range(4):
                        nc.tensor.matmul(
                            ps[:],
                            lhsT=ao[gi][:, blk * 128 : (blk + 1) * 128],
                            rhs=wsb["wo"][:, gi, :],
                            start=(gi == 0),
                            stop=(gi == 3),
                        )
                    osb = opool.tile([128, 512], f32, tag="osb")
                    nc.vector.tensor_copy(out=osb[:], in_=ps[:])
                    r0 = sc * 512 + blk * 128
                    nc.sync.dma_start(out=out_d.ap()[r0 : r0 + 128, :], in_=osb[:])

    nc.compile()
    return nc


def _get_compiled():
    global _compiled
    if _compiled is None:
        _compiled = _build()
    return _compiled


def _prep_in_maps(query, key, value, attn_mask, in_proj_weight):
    import ml_dtypes

    bf = ml_dtypes.bfloat16
    # bf16 transfer: identical rounding to the on-device cast-DMA the
    # kernel previously performed; the device consumes bf16 either way.
    q_t = np.ascontiguousarray(query.transpose(1, 2, 0).astype(bf))   # [B, E, S]
    k_t = np.ascontiguousarray(key.transpose(1, 2, 0).astype(bf))
    v_t = np.ascontiguousarray(value.transpose(1, 2, 0).astype(bf))
    m_t = np.ascontiguousarray(attn_mask.transpose(0, 2, 1).astype(bf))  # [B,T,S]
    # 1/sqrt(D) = 1/8 folded into Wq -- exact in fp32 (power of two)
    wq_t = np.ascontiguousarray((in_proj_weight[0:E] * 0.125).T.astype(bf))
    wk_t = np.ascontiguousarray(in_proj_weight[E : 2 * E].T.astype(bf))
    wv_t = np.ascontiguousarray(in_proj_weight[2 * E : 3 * E].T.astype(bf))
    in_maps = []
    for c in range(NCORES):
        b, hf = c // 2, c % 2
        sl = slice(hf * SC, (hf + 1) * SC)
        in_maps.append(
            {
                "xq_t": np.ascontiguousarray(q_t[b][:, sl]),
                "xk_t": k_t[b],
                "xv_t": v_t[b],
                "mask_t": np.ascontiguousarray(m_t[b][:, sl]),
                "wq_t": wq_t,
                "wk_t": wk_t,
                "wv_t": wv_t,
            }
        )
    return in_maps


def kernel(
    query,
    key,
    value,
    attn_mask,
    in_proj_weight,
    in_proj_bias,
    out_proj_weight,
    out_proj_bias,
):
    from concourse.bass_utils import run_bass_kernel_spmd

    query = np.asarray(query, np.float32)
    key = np.asarray(key, np.float32)
    value = np.asarray(value, np.float32)
    attn_mask = np.asarray(attn_mask, np.float32)
    in_proj_weight = np.asarray(in_proj_weight, np.float32)
    out_proj_weight = np.asarray(out_proj_weight, np.float32)
    out_proj_bias = np.asarray(out_proj_bias, np.float32)

    nc = _get_compiled()
    in_maps = _prep_in_maps(query, key, value, attn_mask, in_proj_weight)
    import ml_dtypes

    wo_t = np.ascontiguousarray(out_proj_weight.T.astype(ml_dtypes.bfloat16))
    for m in in_maps:
        m["wo_t"] = wo_t

    res = run_bass_kernel_spmd(nc, in_maps, core_ids=list(range(NCORES)))

    out = np.empty((S, B, E), np.float32)
    for c in range(NCORES):
        b, hf = c // 2, c % 2
        out[hf * SC : (hf + 1) * SC, b, :] = res.results[c]["out"]
    # out_proj_bias is zeros per the problem spec; adding it on the host is
    # exact. (in_proj biases are also zeros and are not applied on-device.)
    out += out_proj_bias[None, None, :]
    return out


# revision 13
# speedup vs baseline: 1.4221x; 1.0308x over previous
"""Trainium2 Bass kernel for nn_ContentMultiheadAttention_523986010170.

Full (unsharded) inputs in, full output out. Internally shards across 8
NeuronCores: core c handles batch b = c//2 and query-row half c%2 (1024 of
2048 rows), computing all 8 heads for its slice. Outputs are disjoint
[1024, 512] blocks of the [S, B, E] result, gathered on the host.

Device-side math (per core), all matmuls in bf16 with fp32 PSUM accumulation:
  qT = (Wq/8)^T x_q^T        [512, 1024]  (1/sqrt(D) folded into Wq, exact /8)
  kT = Wk^T x_k^T            [512, 2048]
  vN = x_v Wv^T              [2048, 512]  (natural [t, d] layout)
  scoresT_h = kT_h^T qT_h    [t, s] per head (K=64 row-packed head pairs)
  A_h = exp(scoresT_h) * exp(maskT)      (additive mask applied via the
                                          exp factorization; softmax max-
                                          subtraction skipped -- scores are
                                          O(1) so exp cannot overflow)
  outT_h = vN_h^T A_h        (col-packed M=64 head pairs)
  r_h    = ones64^T A_h      (rowsums, already replicated across 64 rows)
  out = (outT * recip(r))^T @ Wo^T   via out-proj matmul

The t-loop is software-pipelined: AV/rowsum matmuls for t-block i are
emitted after the QK matmuls for t-block i+1 so the PE never head-of-line
blocks on the exp/mask-mul round trip.

Host-side work is limited to layout (transpose/slice/concat), the exact
power-of-two weight prescale, and adding out_proj_bias (a zero vector per
the problem spec; in_proj biases are likewise zero and are not applied).
"""

import numpy as np

S, B, E = 2048, 4, 512
H, D = 8, 64
NCORES = 8
SC = S // 2          # query rows per core
T = S                # key rows (full)
NT = T // 128        # t-blocks of 128
NSC = SC // 512      # s-chunks of 512 per core
KC = E // 128        # contraction chunks for projections

_compiled = None


def _build():
    import concourse.bacc as bacc
    import concourse.mybir as mybir
    import concourse.tile as tile

    f32 = mybir.dt.float32
    bf16 = mybir.dt.bfloat16
    Exp = mybir.ActivationFunctionType.Exp

    nc = bacc.Bacc("TRN2", target_bir_lowering=False, debug=False)

    xq_d = nc.dram_tensor("xq_t", [E, SC], bf16, kind="ExternalInput")
    xk_d = nc.dram_tensor("xk_t", [E, T], bf16, kind="ExternalInput")
    xv_d = nc.dram_tensor("xv_t", [E, T], bf16, kind="ExternalInput")
    mask_d = nc.dram_tensor("mask_t", [T, SC], bf16, kind="ExternalInput")
    wq_d = nc.dram_tensor("wq_t", [E, E], bf16, kind="ExternalInput")
    wk_d = nc.dram_tensor("wk_t", [E, E], bf16, kind="ExternalInput")
    wv_d = nc.dram_tensor("wv_t", [E, E], bf16, kind="ExternalInput")
    wo_d = nc.dram_tensor("wo_t", [E, E], bf16, kind="ExternalInput")
    out_d = nc.dram_tensor("out", [SC, E], f32, kind="ExternalOutput")

    with tile.TileContext(nc) as tc:
        with (
            tc.tile_pool(name="persist", bufs=1) as pp,
            tc.tile_pool(name="mstage", bufs=2) as mstage,
            tc.tile_pool(name="work", bufs=4) as wk,
            tc.tile_pool(name="attn", bufs=10) as apool,
            tc.tile_pool(name="aout", bufs=8) as aopool,
            tc.tile_pool(name="outp", bufs=3) as opool,
            tc.tile_pool(name="ps_small", bufs=4, space="PSUM") as ps_s,
            tc.tile_pool(name="ps_scores", bufs=2, space="PSUM") as ps_b,
        ):
            # ---- constants ----
            ones64 = pp.tile([128, 64], bf16, tag="ones64")
            nc.vector.memset(ones64[:], 1)

            # ---- input loads (bf16, HWDGE), ordered by first use ----
            wsb = {}
            for nm, wd in (("wq", wq_d), ("wk", wk_d), ("wv", wv_d), ("wo", wo_d)):
                t_ = pp.tile([128, KC, E], bf16, tag=nm, name=nm)
                nc.sync.dma_start(
                    out=t_[:], in_=wd.ap().rearrange("(c p) e -> p c e", p=128)
                )
                wsb[nm] = t_
            xq = pp.tile([128, KC, SC], bf16, tag="xq")
            nc.sync.dma_start(
                out=xq[:], in_=xq_d.ap().rearrange("(c p) s -> p c s", p=128)
            )
            xk = pp.tile([128, KC, T], bf16, tag="xk")
            nc.sync.dma_start(
                out=xk[:], in_=xk_d.ap().rearrange("(c p) s -> p c s", p=128)
            )
            xv = pp.tile([128, KC, T], bf16, tag="xv")
            nc.sync.dma_start(
                out=xv[:], in_=xv_d.ap().rearrange("(c p) s -> p c s", p=128)
            )

            # ---- mask -> G = exp(mask), bf16, [128, NT, SC] ----
            g = pp.tile([128, NT, SC], bf16, tag="g")
            for i in range(4):  # 4 chunks of 4 t-blocks each
                ms = mstage.tile([128, NT // 4, SC], bf16, tag="mstage", name="ms")
                nc.sync.dma_start(
                    out=ms[:],
                    in_=mask_d.ap()
                    .rearrange("(c p) s -> p c s", p=128)[:, i * 4 : (i + 1) * 4, :],
                )
                nc.scalar.activation(g[:, i * 4 : (i + 1) * 4, :], ms[:], Exp)

            # ---- projections ----
            qT = [
                pp.tile([128, SC], bf16, tag=f"qT{eo}", name=f"qT{eo}")
                for eo in range(KC)
            ]
            kT = [
                pp.tile([128, T], bf16, tag=f"kT{eo}", name=f"kT{eo}")
                for eo in range(KC)
            ]
            vN = [
                pp.tile([128, E], bf16, tag=f"vN{tb}", name=f"vN{tb}")
                for tb in range(NT)
            ]

            for eo in range(KC):
                for c2 in range(SC // 512):
                    ps = ps_s.tile([128, 512], f32, tag="ps_small")
                    for kc in range(KC):
                        nc.tensor.matmul(
                            ps[:],
                            lhsT=wsb["wq"][:, kc, eo * 128 : (eo + 1) * 128],
                            rhs=xq[:, kc, c2 * 512 : (c2 + 1) * 512],
                            start=(kc == 0),
                            stop=(kc == KC - 1),
                        )
                    nc.vector.tensor_copy(
                        out=qT[eo][:, c2 * 512 : (c2 + 1) * 512], in_=ps[:]
                    )
            for eo in range(KC):
                for c2 in range(T // 512):
                    ps = ps_s.tile([128, 512], f32, tag="ps_small")
                    for kc in range(KC):
                        nc.tensor.matmul(
                            ps[:],
                            lhsT=wsb["wk"][:, kc, eo * 128 : (eo + 1) * 128],
                            rhs=xk[:, kc, c2 * 512 : (c2 + 1) * 512],
                            start=(kc == 0),
                            stop=(kc == KC - 1),
                        )
                    nc.vector.tensor_copy(
                        out=kT[eo][:, c2 * 512 : (c2 + 1) * 512], in_=ps[:]
                    )
            for tb in range(NT):
                ps = ps_s.tile([128, 512], f32, tag="ps_small")
                for kc in range(KC):
                    nc.tensor.matmul(
                        ps[:],
                        lhsT=xv[:, kc, tb * 128 : (tb + 1) * 128],
                        rhs=wsb["wv"][:, kc, :],
                        start=(kc == 0),
                        stop=(kc == KC - 1),
                    )
                nc.vector.tensor_copy(out=vN[tb][:], in_=ps[:])

            # ---- attention ----
            for sc in range(NSC):
                ssl = slice(sc * 512, (sc + 1) * 512)
                ao = []  # normalized outT per head-pair, [128, 512] bf16
                for ph in range(2):  # head-groups of 4 (2 pairs)
                    av = [
                        ps_s.tile([128, 512], f32, tag="ps_small", name=f"av{i}")
                        for i in range(2)
                    ]
                    rsp = [
                        ps_s.tile([128, 512], f32, tag="ps_small", name=f"rs{i}")
                        for i in range(2)
                    ]

                    def emit_av_rs(tb, a4):
                        st, sp_ = (tb == 0), (tb == NT - 1)
                        for pr in range(2):
                            for j in range(2):
                                h = (ph * 2 + pr) * 2 + j
                                nc.tensor.matmul(
                                    av[pr][j * 64 : (j + 1) * 64, :],
                                    lhsT=vN[tb][:, h * 64 : (h + 1) * 64],
                                    rhs=a4[pr * 2 + j][:],
                                    start=st,
                                    stop=sp_,
                                    tile_position=(0, j * 64),
                                    skip_group_check=True,
                                )
                        for pr in range(2):
                            for j in range(2):
                                nc.tensor.matmul(
                                    rsp[pr][j * 64 : (j + 1) * 64, :],
                                    lhsT=ones64[:],
                                    rhs=a4[pr * 2 + j][:],
                                    start=st,
                                    stop=sp_,
                                    tile_position=(0, j * 64),
                                    skip_group_check=True,
                                )

                    prev_a = None
                    for tb in range(NT):
                        cur_a = []
                        for pr in range(2):
                            gp = ph * 2 + pr
                            sp = ps_b.tile([128, 1024], f32, tag="ps_scores")
                            for j in range(2):
                                nc.tensor.matmul(
                                    sp[:, j * 512 : (j + 1) * 512],
                                    lhsT=kT[gp][
                                        j * 64 : (j + 1) * 64,
                                        tb * 128 : (tb + 1) * 128,
                                    ],
                                    rhs=qT[gp][j * 64 : (j + 1) * 64, ssl],
                                    start=True,
                                    stop=True,
                                    tile_position=(j * 64, 0),
                                )
                            et = wk.tile([128, 1024], bf16, tag="e", bufs=6)
                            nc.scalar.activation(et[:], sp[:], Exp)
                            for j in range(2):
                                a = apool.tile([128, 512], bf16, tag="a")
                                nc.vector.tensor_mul(
                                    out=a[:],
                                    in0=et[:, j * 512 : (j + 1) * 512],
                                    in1=g[:, tb, ssl],
                                )
                                cur_a.append(a)
                        if prev_a is not None:
                            emit_av_rs(tb - 1, prev_a)
                        prev_a = cur_a
                    emit_av_rs(NT - 1, prev_a)

                    # normalize: recip of (replicated) rowsums, fused mul+cast
                    for pr in range(2):
                        rep = wk.tile([128, 512], f32, tag="rep")
                        nc.vector.reciprocal_approx_fast(
                            out=rep[:], in_=rsp[pr][:]
                        )
                        o = aopool.tile([128, 512], bf16, tag="ao")
                        nc.vector.tensor_mul(out=o[:], in0=av[pr][:], in1=rep[:])
                        ao.append(o)

                # out-proj for this s-chunk
                for blk in range(4):
                    ps = ps_s.tile([128, 512], f32, tag="ps_small")
                    for gi in range(4):
                        nc.tensor.matmul(
                            ps[:],
                            lhsT=ao[gi][:, blk * 128 : (blk + 1) * 128],
                            rhs=wsb["wo"][:, gi, :],
                            start=(gi == 0),
                            stop=(gi == 3),
                        )
                    osb = opool.tile([128, 512], f32, tag="osb")
                    nc.vector.tensor_copy(out=osb[:], in_=ps[:])
                    r0 = sc * 512 + blk * 128
                    nc.sync.dma_start(out=out_d.ap()[r0 : r0 + 128, :], in_=osb[:])

    nc.compile()
    return nc


def _get_compiled():
    global _compiled
    if _compiled is None:
        _compiled = _build()
    return _compiled


def _prep_in_maps(query, key, value, attn_mask, in_proj_weight):
    import ml_dtypes

    bf = ml_dtypes.bfloat16
    # bf16 transfer: identical rounding to the on-device cast-DMA the
    # kernel previously performed; the device consumes bf16 either way.
    q_t = np.ascontiguousarray(query.transpose(1, 2, 0).astype(bf))   # [B, E, S]
    k_t = np.ascontiguousarray(key.transpose(1, 2, 0).astype(bf))
    v_t = np.ascontiguousarray(value.transpose(1, 2, 0).astype(bf))
    m_t = np.ascontiguousarray(attn_mask.transpose(0, 2, 1).astype(bf))  # [B,T,S]
    # 1/sqrt(D) = 1/8 folded into Wq -- exact in fp32 (power of two)
    wq_t = np.ascontiguousarray((in_proj_weight[0:E] * 0.125).T.astype(bf))
    wk_t = np.ascontiguousarray(in_proj_weight[E : 2 * E].T.astype(bf))
    wv_t = np.ascontiguousarray(in_proj_weight[2 * E : 3 * E].T.astype(bf))
    in_maps = []
    for c in range(NCORES):
        b, hf = c // 2, c % 2
        sl = slice(hf * SC, (hf + 1) * SC)
        in_maps.append(
            {
                "xq_t": np.ascontiguousarray(q_t[b][:, sl]),
                "xk_t": k_t[b],
                "xv_t": v_t[b],
                "mask_t": np.ascontiguousarray(m_t[b][:, sl]),
                "wq_t": wq_t,
                "wk_t": wk_t,
                "wv_t": wv_t,
            }
        )
    return in_maps


def kernel(
    query,
    key,
    value,
    attn_mask,
    in_proj_weight,
    in_proj_bias,
    out_proj_weight,
    out_proj_bias,
):
    from concourse.bass_utils import run_bass_kernel_spmd

    query = np.asarray(query, np.float32)
    key = np.asarray(key, np.float32)
    value = np.asarray(value, np.float32)
    attn_mask = np.asarray(attn_mask, np.float32)
    in_proj_weight = np.asarray(in_proj_weight, np.float32)
    out_proj_weight = np.asarray(out_proj_weight, np.float32)
    out_proj_bias = np.asarray(out_proj_bias, np.float32)

    nc = _get_compiled()
    in_maps = _prep_in_maps(query, key, value, attn_mask, in_proj_weight)
    import ml_dtypes

    wo_t = np.ascontiguousarray(out_proj_weight.T.astype(ml_dtypes.bfloat16))
    for m in in_maps:
        m["wo_t"] = wo_t

    res = run_bass_kernel_spmd(nc, in_maps, core_ids=list(range(NCORES)))

    out = np.empty((S, B, E), np.float32)
    for c in range(NCORES):
        b, hf = c // 2, c % 2
        out[hf * SC : (hf + 1) * SC, b, :] = res.results[c]["out"]
    # out_proj_bias is zeros per the problem spec; adding it on the host is
    # exact. (in_proj biases are also zeros and are not applied on-device.)
    out += out_proj_bias[None, None, :]
    return out
